# revision 61
# baseline (speedup 1.0000x reference)
"""Trainium2 Bass kernel for a 2-layer LSTMCell autoencoder (batch=1).

Reference computation:
    h1, c1 = LSTMCell1(x, (h_t, c_t))      # input 4000 -> hidden 5000
    h2, c2 = LSTMCell2(h1, (h2_t, c2_t))   # hidden 5000 -> hidden 5000
    out = h2 @ w_lin.T + b_lin             # hidden 5000 -> 4000

Strategy (8 NeuronCores):
  - Cell 1 is column-sharded: core r computes gate slice r (625 of each
    gate) of cell 1 from the replicated x, entirely locally.
  - Cell 2 and the final linear are CONTRACTION-sharded: core r's weight
    slab holds the rows multiplied by its own locally-computed h slice,
    so no AllGather is ever needed.  The per-core partial gate / output
    vectors are exchanged with two small AllToAlls (the only
    collectives, both after the heavy compute) and summed locally with
    a ones-vector matmul into PSUM.  (ReduceScatter would express this
    directly but measures ~8x slower than bypass collectives here.)
  - All matvecs run as psum[1,N] += vec[128,1].T @ W[128,N] with the
    weights streamed from HBM as the moving operand, k-major so
    consecutive matmuls hit different PSUM banks.
  - Biases fold in as an extra weight row against a 1.0 vec element
    (nonzero only on core 0 for the contraction-sharded layers).
  - Fast path (state vectors h_t/c_t/h2_t/c2_t all zero, which is how
    the module's persistent buffers are initialized): the h@W_hh rows
    are exactly zero and are not streamed, and the f gate is unused
    (c_new = sig(f)*0 + sig(i)*tanh(g)) so its columns are skipped too.
    A general graph with the full row/column set is compiled lazily if
    any state is nonzero.
  - Weights are stored in HBM as int8 (symmetric per-matrix scale) and
    dequantized to bf16 by the DMA engines on the way into SBUF (SWDGE
    dtype-cast), halving HBM traffic at ~int8 accuracy.  The descale
    rides the activation / psum-copy instructions' scale operand.
  - w_lin is fully prefetched into SBUF so the final matmuls never wait
    on DMA; h vectors change layout via direct SBUF->SBUF DMA (no DRAM
    round trip).

kernel(**inputs) takes the full unsharded inputs, returns full output.
"""
import sys

sys.path.insert(0, "/opt/trn_rl_repo")

import ml_dtypes
import numpy as np

import concourse.bacc as bacc
import concourse.tile as tile
import concourse.mybir as mybir
from concourse.bass_utils import run_bass_kernel_spmd

N_CORES = 8
I_DIM = 4000
H_DIM = 5000
HS = H_DIM // N_CORES          # 625 per-core slice of each gate
OS = I_DIM // N_CORES          # 500 output slice per core
SEG = 640                      # padded h slice (625 + 1.0 slot + pad)
XSEG = 4096                    # x(4000) + 1.0 + pad
HSEG = 5120                    # h_t(5000) + pad

DT = mybir.dt.float32
VDT = mybir.dt.bfloat16        # vec (moving-operand partner) dtype
F32 = np.float32
BF16 = ml_dtypes.bfloat16

# Weight mode: "i8" = int8 in HBM, DMA-cast to bf16 in SBUF (descale via
# activation scale); "bf16" = plain bf16 end to end.
WMODE = "i8"

Sig = mybir.ActivationFunctionType.Sigmoid
Tanh = mybir.ActivationFunctionType.Tanh
Copy = mybir.ActivationFunctionType.Copy

_CACHED = {}


def _chunks(total, step=500):
    return [(c, min(c + step, total)) for c in range(0, total, step)]


def _cfg(fast):
    """Geometry for the two graph variants."""
    gates = "igo" if fast else "ifgo"
    ng = len(gates)
    gpc = ng * HS              # gate cols per core (cell1) / per rank (cell2)
    return dict(
        gates=gates,
        gpc=gpc,
        G=gpc * N_CORES,       # total cell-2 gate columns
        r1=XSEG if fast else XSEG + HSEG,
        r2=SEG if fast else 2 * SEG,
        cw2=2500 if fast else 1250,   # cell-2 column window per weight tile
        ns=HS * N_CORES,              # cell-2 columns per gate split (5000)
        # cell-1 psum chunks: widths divisible by ng (gates interleave)
        c1_chunks=_chunks(gpc, (500 // ng) * ng),
        cl_chunks=_chunks(I_DIM),
    )


def _act_map(gates, chunks):
    """(chunk_idx, chunk-local lo, hi, func, global lo, hi) covering the
    gate vector laid out [g0|g1|...] x HS."""
    funcs = {"i": Sig, "f": Sig, "g": Tanh, "o": Sig}
    amap = []
    for gi, gname in enumerate(gates):
        glo, ghi = gi * HS, (gi + 1) * HS
        for ci, (c0, c1) in enumerate(chunks):
            lo, hi = max(glo, c0), min(ghi, c1)
            if lo < hi:
                amap.append((ci, lo - c0, hi - c0, funcs[gname], lo, hi))
    return amap


def _build_bass(fast):
    cfg = _cfg(fast)
    GPC, G, CW = cfg["gpc"], cfg["G"], cfg["cw2"]
    B1 = cfg["r1"] // 128
    B2 = cfg["r2"] // 128
    BL = SEG // 128            # 5 k-blocks for the final linear

    i8 = WMODE == "i8"
    wdt_dram = mybir.dt.int8 if i8 else VDT

    nc = bacc.Bacc("TRN2", target_bir_lowering=False, debug=False,
                   num_devices=N_CORES)

    w1_ext = nc.dram_tensor("w1", [cfg["r1"], GPC], wdt_dram, kind="ExternalInput")
    w2_ext = nc.dram_tensor("w2", [cfg["r2"], G], wdt_dram, kind="ExternalInput")
    wl_ext = nc.dram_tensor("wl", [SEG, I_DIM], wdt_dram, kind="ExternalInput")
    vec1_ext = nc.dram_tensor("vec1", [128, B1], VDT, kind="ExternalInput")
    if not fast:
        h2ts_ext = nc.dram_tensor("h2ts", [128, BL], VDT, kind="ExternalInput")
        # c states replicated across partitions (any column group reads)
        c1s_ext = nc.dram_tensor("c1s", [128, HS], DT, kind="ExternalInput")
        c2s_ext = nc.dram_tensor("c2s", [128, HS], DT, kind="ExternalInput")
    # per-matrix dequant scales (1.0 in bf16 mode): [s1, s2, sl, 0],
    # replicated across partitions so any column group can read them
    sc_ext = nc.dram_tensor("sc", [128, 4], DT, kind="ExternalInput")
    out_ext = nc.dram_tensor("out", [1, OS], DT, kind="ExternalOutput")

    NS = cfg["ns"]                      # per-gate split width (8 x 625)
    NSPLIT = len(cfg["gates"])
    h1b = nc.dram_tensor("h1b", [SEG], VDT)
    h2b = nc.dram_tensor("h2b", [SEG], VDT)
    g2p = [nc.dram_tensor(f"g2p{s}", [NS], VDT) for s in range(NSPLIT)]
    g2q = [nc.dram_tensor(f"g2q{s}", [NS], VDT) for s in range(NSPLIT)]
    warm_in = nc.dram_tensor("warm_in", [64], VDT)
    warm_out = nc.dram_tensor("warm_out", [64], VDT)
    opart = nc.dram_tensor("opart", [I_DIM], DT)
    oa2a = nc.dram_tensor("oa2a", [I_DIM], DT)

    groups = [list(range(N_CORES))]

    with tile.TileContext(nc) as tc:
        with (
            tc.tile_pool(name="w1pool", bufs=3 if fast else 2) as wpool,
            tc.tile_pool(name="w2pool", bufs=2) as w2pool,
            tc.tile_pool(name="wlpool", bufs=1) as wlpool,
            tc.tile_pool(name="misc", bufs=1) as misc,
            tc.tile_pool(name="stage", bufs=2) as stpool,
            tc.tile_pool(name="psum", bufs=8, space="PSUM") as ppool,
        ):
            hwdge = [nc.sync, nc.scalar]
            small_i = 0

            def wdma(dst, src):
                # weight streams: SWDGE (casting) in i8 mode, HWDGE else
                nonlocal small_i
                if i8:
                    nc.gpsimd.dma_start(out=dst, in_=src)
                else:
                    hwdge[small_i % 2].dma_start(out=dst, in_=src)
                    small_i += 1

            def sdma(dst, src):
                # small transfers: HWDGE in i8 mode (queues are free),
                # SWDGE otherwise (HWDGE busy with weights)
                nonlocal small_i
                if i8:
                    hwdge[small_i % 2].dma_start(out=dst, in_=src)
                    small_i += 1
                else:
                    nc.gpsimd.dma_start(out=dst, in_=src)

            # --- small input DMAs ---
            vec1_sb = misc.tile([128, B1], VDT, name="vec1sb")
            nc.gpsimd.dma_start(out=vec1_sb[:], in_=vec1_ext[:])
            vec2_sb = misc.tile([128, B2], VDT, name="vec2sb")
            if not fast:
                nc.gpsimd.dma_start(out=vec2_sb[:, BL:B2], in_=h2ts_ext[:])
            vecl_sb = misc.tile([128, BL], VDT, name="veclsb")
            sc_sb = misc.tile([128, 4], DT, name="scsb")
            nc.gpsimd.dma_start(out=sc_sb[:], in_=sc_ext[:])
            ones_sb = misc.tile([N_CORES, 1], DT, name="onessb")
            nc.vector.memset(ones_sb[:], 1.0)
            ones_bf = misc.tile([N_CORES, 1], VDT, name="onesbf")
            nc.vector.memset(ones_bf[:], 1.0)
            # tiny warmup collective: pays the collectives-path first-use
            # cost (~15us) inside the init-barrier shadow
            warm_sb = misc.tile([1, 64], VDT, name="warmsb")
            nc.vector.memset(warm_sb[:], 0.0)
            nc.gpsimd.dma_start(out=warm_in.ap(), in_=warm_sb[0:1, :])
            nc.gpsimd.collective_compute(
                "AllToAll", mybir.AluOpType.bypass, replica_groups=groups,
                ins=[warm_in.ap().opt()], outs=[warm_out.ap().opt()])
            c1_sb = c2_sb = None
            if not fast:
                c1_sb = misc.tile([128, HS], DT, name="c1sb")
                c2_sb = misc.tile([128, HS], DT, name="c2sb")
                nc.gpsimd.dma_start(out=c1_sb[:], in_=c1s_ext[:])
                nc.gpsimd.dma_start(out=c2_sb[:], in_=c2s_ext[:])

            def to_vec(hpad_sb, dram, vdst):
                """[1, 640] partition-0 h vector -> [128, 5] vec layout,
                bounced through DRAM."""
                sdma(dram.ap(), hpad_sb[0:1, :])
                sdma(vdst, dram.ap().rearrange("(b p) -> p b", p=128))

            # =========== cell 1: column-sharded, all local ===========
            # Gate columns are element-interleaved (i,g,o of one element
            # adjacent), so every psum chunk holds complete gate tuples:
            # activations and the c/h elementwise math run per chunk AT
            # THAT CHUNK'S PARTITION with stride-ng APs, and finished h
            # chunks store straight to the bounce buffer -- no partition-0
            # reassembly.  The psum chunks ride different PE column groups
            # (tile_position) so each k-block's matmuls run concurrently.
            gs = cfg["gates"]
            ng = len(gs)
            funcs = {"i": Sig, "f": Sig, "g": Tanh, "o": Sig}
            chunks1 = cfg["c1_chunks"]
            pg = [ppool.tile([128, 512], DT, name=f"pg{h}", tag="ps")
                  for h in range((len(chunks1) + 3) // 4)]
            # small first tile so the tensor engine starts early
            tiles1 = [(0, 2)] + [(b, min(b + 6, B1) - b) for b in range(2, B1, 6)]
            for ti, (b0, nb) in enumerate(tiles1):
                wt = wpool.tile([128, 6, GPC], VDT, tag="w", name="w1t")
                wdma(wt[:, 0:nb, :],
                     w1_ext[b0 * 128:(b0 + nb) * 128, :]
                     .rearrange("(n p) c -> p n c", p=128))
                for j in range(nb):
                    b = b0 + j
                    for n, (c0, c1) in enumerate(chunks1):
                        cg = 32 * (n % 4)
                        nc.tensor.matmul(
                            pg[n // 4][cg:cg + 1, 0:c1 - c0],
                            vec1_sb[:, b:b + 1],
                            wt[:, j, c0:c1],
                            start=(b == 0), stop=(b == B1 - 1),
                            tile_position=(0, cg))
            sg1 = stpool.tile([128, 1024], DT, name="sg1", tag="sg")
            htmp = misc.tile([128, 256], DT, name="htmp")
            hout = misc.tile([128, 256], VDT, name="hout")
            for n, (c0, c1) in enumerate(chunks1):
                cg = 32 * (n % 4)
                co = (n // 4) * 512
                w = c1 - c0
                ne = w // ng
                e0 = c0 // ng
                for t, gname in enumerate(gs):
                    nc.scalar.activation(
                        sg1[cg:cg + 1, co + t:co + w:ng],
                        pg[n // 4][cg:cg + 1, t:w:ng],
                        funcs[gname], scale=sc_sb[cg:cg + 1, 0:1])
                sl = lambda ch: sg1[cg:cg + 1, co + gs.index(ch):co + w:ng]
                m1 = htmp[cg:cg + 1, 0:ne]
                nc.vector.tensor_mul(m1, sl("i"), sl("g"))
                if not fast:
                    m2 = htmp[cg:cg + 1, 128:128 + ne]
                    nc.vector.tensor_mul(m2, sl("f"),
                                         c1_sb[cg:cg + 1, e0:e0 + ne])
                    nc.vector.tensor_add(m1, m1, m2)
                nc.scalar.activation(m1, m1, Tanh)
                ho = hout[cg:cg + 1, 0:ne]
                nc.vector.tensor_mul(ho, sl("o"), m1)
                sdma(h1b[e0:e0 + ne], ho)
            hpadc = misc.tile([1, SEG - HS], VDT, name="hpadc")
            nc.vector.memset(hpadc[:], 0.0)
            nc.vector.memset(hpadc[:, 0:1], 1.0)
            sdma(h1b[HS:SEG], hpadc[0:1, :])
            sdma(vec2_sb[:, 0:BL],
                 h1b.ap().rearrange("(b p) -> p b", p=128))

            # w_lin prefetch: queued here (no buffer-reuse waits of its
            # own) so the gpsimd queue reaches the collective triggers
            # without waiting on it
            wl_sb = wlpool.tile([128, BL, I_DIM], VDT, name="wlsb")
            wdma(wl_sb[:],
                 wl_ext.ap().rearrange("(n p) c -> p n c", p=128))

            # ==== cell 2: contraction-sharded, partial sums over all G ====
            # Columns are laid out gate-major then rank-major, so each
            # per-gate group of NS columns feeds its own AllToAll as soon
            # as its tiles' stores land (the first collective's ~12us
            # setup hides behind the remaining weight stream).  Weight
            # tiles span all B2 k-blocks x a column window; chunks within
            # a tile are k-major (PSUM-bank-alternating) and staged out
            # as one batch store per tile.
            for t0 in range(0, G, CW):
                wt = w2pool.tile([128, B2, CW], VDT, tag="w2", name="w2t")
                wdma(wt[:],
                     w2_ext[:, t0:t0 + CW]
                     .rearrange("(n p) c -> p n c", p=128))
                tchunks = _chunks(CW)
                pc = [ppool.tile([128, 512], DT, name=f"pc{h}", tag="ps")
                      for h in range((len(tchunks) + 3) // 4)]
                for b in range(B2):
                    for n, (c0, c1) in enumerate(tchunks):
                        cg = 32 * (n % 4)
                        nc.tensor.matmul(
                            pc[n // 4][cg:cg + 1, 0:c1 - c0],
                            vec2_sb[:, b:b + 1],
                            wt[:, b, c0:c1],
                            start=(b == 0), stop=(b == B2 - 1),
                            tile_position=(0, cg))
                st = stpool.tile([128, 1024], VDT, name="st", tag="sg2")
                for n, (c0, c1) in enumerate(tchunks):
                    cg = 32 * (n % 4)
                    co = (n // 4) * 512
                    nc.scalar.activation(st[cg:cg + 1, co:co + c1 - c0],
                                         pc[n // 4][cg:cg + 1, 0:c1 - c0],
                                         Copy, scale=sc_sb[cg:cg + 1, 1:2])
                    sdma(g2p[t0 // NS][t0 % NS + c0:t0 % NS + c1],
                         st[cg:cg + 1, co:co + c1 - c0])

            # --- per-gate AllToAll + local ones-matmul reduction; the
            # c/h elementwise math interleaves between splits ---
            g2a = misc.tile([1, GPC], DT, name="g2a")
            h2pad = misc.tile([1, SEG], VDT, name="h2pad")
            nc.vector.memset(h2pad[:], 0.0)
            nc.vector.memset(h2pad[:, HS:HS + 1], 1.0)
            m1 = misc.tile([1, HS], DT, name="m1c2")
            funcs = {"i": Sig, "f": Sig, "g": Tanh, "o": Sig}
            for s, gname in enumerate(cfg["gates"]):
                nc.gpsimd.collective_compute(
                    "AllToAll", mybir.AluOpType.bypass, replica_groups=groups,
                    ins=[g2p[s].ap().opt()], outs=[g2q[s].ap().opt()])
                parts = misc.tile([N_CORES, HS], VDT, name=f"parts{s}")
                sdma(parts[:], g2q[s].ap().rearrange("(q e) -> q e", q=N_CORES))
                ga = g2a[:, s * HS:(s + 1) * HS]
                for (c0, c1) in _chunks(HS):
                    pr = ppool.tile([1, 512], DT, name=f"pr{s}", tag="ps")
                    nc.tensor.matmul(pr[:, 0:c1 - c0], ones_bf[:],
                                     parts[:, c0:c1], start=True, stop=True)
                    nc.scalar.activation(ga[:, c0:c1], pr[:, 0:c1 - c0],
                                         funcs[gname])
                # interleaved h math once inputs are ready
                g = cfg["gates"]
                if gname == "g":
                    ap = lambda ch: g2a[:, g.index(ch) * HS:
                                        (g.index(ch) + 1) * HS]
                    nc.vector.tensor_mul(m1[:], ap("i"), ap("g"))
                    if not fast:
                        m2 = misc.tile([1, HS], DT, name="m2c2")
                        nc.vector.tensor_mul(m2[:], ap("f"), c2_sb[0:1, :])
                        nc.vector.tensor_add(m1[:], m1[:], m2[:])
                    nc.scalar.activation(m1[:], m1[:], Tanh)
                if gname == "o":
                    o_ap = g2a[:, g.index("o") * HS:(g.index("o") + 1) * HS]
                    nc.vector.tensor_mul(h2pad[:, 0:HS], o_ap, m1[:])
            to_vec(h2pad, h2b, vecl_sb[:])

            # ===== final linear: contraction-sharded, SBUF-resident =====
            # a few throwaway matmuls keep the PE clock warm through the
            # collective wait so the final matvecs run at full rate
            pdum = ppool.tile([1, 512], DT, name="pdum", tag="ps")
            for _ in range(12):
                nc.tensor.matmul(pdum[:, 0:512], wl_sb[:, 0, 0:1],
                                 wl_sb[:, 1, 0:512], start=True, stop=True)
            chunksl = cfg["cl_chunks"]
            pl = [ppool.tile([128, 512], DT, name=f"pl{h}", tag="ps")
                  for h in range((len(chunksl) + 3) // 4)]
            for b in range(BL):
                for n, (c0, c1) in enumerate(chunksl):
                    cg = 32 * (n % 4)
                    nc.tensor.matmul(
                        pl[n // 4][cg:cg + 1, 0:c1 - c0],
                        vecl_sb[:, b:b + 1],
                        wl_sb[:, b, c0:c1],
                        start=(b == 0), stop=(b == BL - 1),
                        tile_position=(0, cg))
            stl = stpool.tile([128, 1024], DT, name="stl", tag="sg")
            for n, (c0, c1) in enumerate(chunksl):
                cg = 32 * (n % 4)
                co = (n // 4) * 512
                nc.scalar.activation(stl[cg:cg + 1, co:co + c1 - c0],
                                     pl[n // 4][cg:cg + 1, 0:c1 - c0],
                                     Copy, scale=sc_sb[cg:cg + 1, 2:3])
                sdma(opart[c0:c1], stl[cg:cg + 1, co:co + c1 - c0])

            # --- AllToAll + local reduction -> this core's output slice ---
            nc.gpsimd.collective_compute(
                "AllToAll", mybir.AluOpType.bypass, replica_groups=groups,
                ins=[opart.ap().opt()], outs=[oa2a.ap().opt()])
            partsl = misc.tile([N_CORES, OS], DT, name="partsl")
            sdma(partsl[:], oa2a.ap().rearrange("(q e) -> q e", q=N_CORES))
            po = ppool.tile([1, 512], DT, name="po", tag="ps")
            nc.tensor.matmul(po[:, 0:OS], ones_sb[:], partsl[:],
                             start=True, stop=True)
            out_sb = misc.tile([1, OS], DT, name="outsb")
            nc.scalar.activation(out_sb[:], po[:, 0:OS], Copy)
            nc.sync.dma_start(out=out_ext[:], in_=out_sb[0:1, :])

    nc.compile()
    return nc, cfg


def _quant(w):
    """Symmetric int8 quantization; returns (stored array, descale)."""
    if WMODE == "i8":
        s = float(np.abs(w).max()) / 127.0
        if s == 0.0:
            s = 1.0
        return np.round(w / s).astype(np.int8), s
    return w.astype(BF16), 1.0


def _gate_cols(w, r, gates):
    """[in_dim, gpc] column block for core r (gate-major), transposed so
    rows are the contraction dim."""
    gidx = {"i": 0, "f": 1, "g": 2, "o": 3}
    outb = np.empty((w.shape[1], len(gates) * HS), dtype=F32)
    for k, gname in enumerate(gates):
        rows = slice(gidx[gname] * H_DIM + r * HS,
                     gidx[gname] * H_DIM + (r + 1) * HS)
        outb[:, k * HS:(k + 1) * HS] = w[rows, :].T
    return outb


def _gate_bias(b_a, b_b, r, gates):
    gidx = {"i": 0, "f": 1, "g": 2, "o": 3}
    out = np.empty((len(gates) * HS,), dtype=F32)
    for k, gname in enumerate(gates):
        rows = slice(gidx[gname] * H_DIM + r * HS,
                     gidx[gname] * H_DIM + (r + 1) * HS)
        out[k * HS:(k + 1) * HS] = b_a[rows] + b_b[rows]
    return out


def _ileave(a, ng):
    """[..., ng*HS] gate-major -> element-interleaved (elem, gate)."""
    shp = a.shape[:-1]
    a = a.reshape(shp + (ng, HS))
    a = np.moveaxis(a, -2, -1)
    return np.ascontiguousarray(a.reshape(shp + (ng * HS,)))


def _perm_gate_major(a, ng):
    """[..., 8*ng*HS] rank-major -> gate-major (gate, rank, elem) layout."""
    shp = a.shape[:-1]
    a = a.reshape(shp + (N_CORES, ng, HS))
    a = np.moveaxis(a, -3, -2)
    return np.ascontiguousarray(a.reshape(shp + (N_CORES * ng * HS,)))


def _prep_shared(fast, cfg, args):
    """Host-side tensors shared across cores (full cell-2 / w_lin column
    panels, gate-major then rank-major; sliced by contraction rows per
    core)."""
    gates = cfg["gates"]
    ng = len(gates)
    cols2 = _perm_gate_major(np.concatenate(
        [_gate_cols(args["w_ih2"], q, gates) for q in range(N_CORES)],
        axis=1), ng)
    bias2 = _perm_gate_major(np.concatenate(
        [_gate_bias(args["b_ih2"], args["b_hh2"], q, gates)
         for q in range(N_CORES)]), ng)
    colsh2 = None
    if not fast:
        colsh2 = _perm_gate_major(np.concatenate(
            [_gate_cols(args["w_hh2"], q, gates) for q in range(N_CORES)],
            axis=1), ng)
    return dict(cols2=cols2, bias2=bias2, colsh2=colsh2,
                wlT=args["w_lin"].T.astype(F32))


def _prep_core(r, fast, cfg, shared, input_data, w_ih1, w_hh1, b_ih1, b_hh1,
               w_ih2, w_hh2, b_ih2, b_hh2, w_lin, b_lin,
               h_t, c_t, h2_t, c2_t):
    gates, GPC, G = cfg["gates"], cfg["gpc"], cfg["G"]

    # --- W1 (column-sharded): [x-seg | (h-seg)] x GPC, columns
    # element-interleaved so each psum chunk holds whole gate tuples ---
    ng = len(gates)
    w1 = np.zeros((cfg["r1"], GPC), dtype=F32)
    w1[0:I_DIM] = _ileave(_gate_cols(w_ih1, r, gates), ng)
    w1[I_DIM] = _ileave(_gate_bias(b_ih1, b_hh1, r, gates), ng)
    if not fast:
        w1[XSEG:XSEG + H_DIM] = _ileave(_gate_cols(w_hh1, r, gates), ng)

    # --- W2 (contraction-sharded): [own h1 rows | (own h2_t rows)] x G ---
    w2 = np.zeros((cfg["r2"], G), dtype=F32)
    w2[0:HS] = shared["cols2"][r * HS:(r + 1) * HS]
    if r == 0:
        w2[HS] = shared["bias2"]          # rides the 1.0 slot, core 0 only
    if not fast:
        w2[SEG:SEG + HS] = shared["colsh2"][r * HS:(r + 1) * HS]

    # --- W_lin (contraction-sharded): [own h2 rows] x I_DIM ---
    wl = np.zeros((SEG, I_DIM), dtype=F32)
    wl[0:HS] = shared["wlT"][r * HS:(r + 1) * HS]
    if r == 0:
        wl[HS] = b_lin

    w1, s1 = _quant(w1)
    w2, s2 = _quant(w2)
    wl, sl = _quant(wl)

    vec1 = np.zeros((cfg["r1"],), dtype=BF16)
    vec1[0:I_DIM] = input_data[0]
    vec1[I_DIM] = 1.0
    if not fast:
        vec1[XSEG:XSEG + H_DIM] = h_t[0]
    vec1 = np.ascontiguousarray(vec1.reshape(cfg["r1"] // 128, 128).T)

    m = {
        "w1": w1, "w2": w2, "wl": wl, "vec1": vec1,
        "sc": np.tile(np.array([[s1, s2, sl, 0.0]], dtype=F32), (128, 1)),
    }
    if not fast:
        h2ts = np.zeros((SEG,), dtype=BF16)
        h2ts[0:HS] = h2_t[0, r * HS:(r + 1) * HS]
        m["h2ts"] = np.ascontiguousarray(h2ts.reshape(SEG // 128, 128).T)
        m["c1s"] = np.tile(c_t[:, r * HS:(r + 1) * HS].astype(F32), (128, 1))
        m["c2s"] = np.tile(c2_t[:, r * HS:(r + 1) * HS].astype(F32), (128, 1))
    return m


def kernel(**inputs):
    args = {k: np.asarray(v, dtype=F32) for k, v in inputs.items()}
    fast = not any(np.any(args[k]) for k in ("h_t", "c_t", "h2_t", "c2_t"))

    if fast not in _CACHED:
        _CACHED[fast] = _build_bass(fast)
    nc, cfg = _CACHED[fast]

    shared = _prep_shared(fast, cfg, args)
    in_maps = [_prep_core(r, fast, cfg, shared, **args) for r in range(N_CORES)]
    res = run_bass_kernel_spmd(nc, in_maps, core_ids=list(range(N_CORES)))
    out = np.concatenate([res.results[r]["out"][0] for r in range(N_CORES)])
    return out.reshape(1, I_DIM).astype(np.float32)


# revision 71
# speedup vs baseline: 1.3699x; 1.3699x over previous
"""Trainium2 Bass kernel for a 2-layer LSTMCell autoencoder (batch=1).

Reference computation:
    h1, c1 = LSTMCell1(x, (h_t, c_t))      # input 4000 -> hidden 5000
    h2, c2 = LSTMCell2(h1, (h2_t, c2_t))   # hidden 5000 -> hidden 5000
    out = h2 @ w_lin.T + b_lin             # hidden 5000 -> 4000

Strategy (8 NeuronCores):
  - Cell 1 is column-sharded: core r computes gate slice r (625 of each
    gate) of cell 1 from the replicated x, entirely locally.
  - Cell 2 and the final linear are CONTRACTION-sharded: core r's weight
    slab holds the rows multiplied by its own locally-computed h slice,
    so no AllGather is ever needed.  The per-core partial gate / output
    vectors are exchanged with two small AllToAlls (the only
    collectives, both after the heavy compute) and summed locally with
    a ones-vector matmul into PSUM.  (ReduceScatter would express this
    directly but measures ~8x slower than bypass collectives here.)
  - All matvecs run as psum[1,N] += vec[128,1].T @ W[128,N] with the
    weights streamed from HBM as the moving operand, k-major so
    consecutive matmuls hit different PSUM banks.
  - Biases fold in as an extra weight row against a 1.0 vec element
    (nonzero only on core 0 for the contraction-sharded layers).
  - Fast path (state vectors h_t/c_t/h2_t/c2_t all zero, which is how
    the module's persistent buffers are initialized): the h@W_hh rows
    are exactly zero and are not streamed, and the f gate is unused
    (c_new = sig(f)*0 + sig(i)*tanh(g)) so its columns are skipped too.
    A general graph with the full row/column set is compiled lazily if
    any state is nonzero.
  - Weights are stored in HBM as int8 (symmetric per-matrix scale) and
    dequantized to bf16 by the DMA engines on the way into SBUF (SWDGE
    dtype-cast), halving HBM traffic at ~int8 accuracy.  The descale
    rides the activation / psum-copy instructions' scale operand.
  - w_lin is fully prefetched into SBUF so the final matmuls never wait
    on DMA; h vectors change layout via direct SBUF->SBUF DMA (no DRAM
    round trip).

kernel(**inputs) takes the full unsharded inputs, returns full output.
"""
import sys

sys.path.insert(0, "/opt/trn_rl_repo")

import ml_dtypes
import numpy as np

import concourse.bacc as bacc
import concourse.tile as tile
import concourse.mybir as mybir
from concourse.bass_utils import run_bass_kernel_spmd

N_CORES = 8
I_DIM = 4000
H_DIM = 5000
HS = H_DIM // N_CORES          # 625 per-core slice of each gate
OS = I_DIM // N_CORES          # 500 output slice per core
SEG = 640                      # padded h slice (625 + 1.0 slot + pad)
XSEG = 4096                    # x(4000) + 1.0 + pad
HSEG = 5120                    # h_t(5000) + pad

DT = mybir.dt.float32
VDT = mybir.dt.bfloat16        # vec (moving-operand partner) dtype
F32 = np.float32
BF16 = ml_dtypes.bfloat16

# Weight mode: "i8" = int8 in HBM, DMA-cast to bf16 in SBUF (descale via
# activation scale); "bf16" = plain bf16 end to end.
WMODE = "i8"

Sig = mybir.ActivationFunctionType.Sigmoid
Tanh = mybir.ActivationFunctionType.Tanh
Copy = mybir.ActivationFunctionType.Copy

_CACHED = {}


def _chunks(total, step=500):
    return [(c, min(c + step, total)) for c in range(0, total, step)]


def _cfg(fast):
    """Geometry for the two graph variants."""
    gates = "igo" if fast else "ifgo"
    gpc = len(gates) * HS      # gate cols per core (cell1) / per rank (cell2)
    return dict(
        gates=gates,
        gpc=gpc,
        G=gpc * N_CORES,       # total cell-2 gate columns
        r1=XSEG if fast else XSEG + HSEG,
        r2=SEG if fast else 2 * SEG,
        cw2=2500 if fast else 1250,   # cell-2 column window per weight tile
        ns=HS * N_CORES,              # cell-2 columns per gate split (5000)
        c1_chunks=_chunks(gpc),
        cl_chunks=_chunks(I_DIM),
    )


def _act_map(gates, chunks):
    """(chunk_idx, chunk-local lo, hi, func, global lo, hi) covering the
    gate vector laid out [g0|g1|...] x HS."""
    funcs = {"i": Sig, "f": Sig, "g": Tanh, "o": Sig}
    amap = []
    for gi, gname in enumerate(gates):
        glo, ghi = gi * HS, (gi + 1) * HS
        for ci, (c0, c1) in enumerate(chunks):
            lo, hi = max(glo, c0), min(ghi, c1)
            if lo < hi:
                amap.append((ci, lo - c0, hi - c0, funcs[gname], lo, hi))
    return amap


def _build_bass(fast):
    cfg = _cfg(fast)
    GPC, G, CW = cfg["gpc"], cfg["G"], cfg["cw2"]
    B1 = cfg["r1"] // 128
    B2 = cfg["r2"] // 128
    BL = SEG // 128            # 5 k-blocks for the final linear

    i8 = WMODE == "i8"
    wdt_dram = mybir.dt.int8 if i8 else VDT

    nc = bacc.Bacc("TRN2", target_bir_lowering=False, debug=False,
                   num_devices=N_CORES)

    w1_ext = nc.dram_tensor("w1", [cfg["r1"], GPC], wdt_dram, kind="ExternalInput")
    w2_ext = nc.dram_tensor("w2", [cfg["r2"], G], wdt_dram, kind="ExternalInput")
    wl_ext = nc.dram_tensor("wl", [SEG, I_DIM], wdt_dram, kind="ExternalInput")
    vec1_ext = nc.dram_tensor("vec1", [128, B1], VDT, kind="ExternalInput")
    if not fast:
        h2ts_ext = nc.dram_tensor("h2ts", [128, BL], VDT, kind="ExternalInput")
        c1s_ext = nc.dram_tensor("c1s", [1, HS], DT, kind="ExternalInput")
        c2s_ext = nc.dram_tensor("c2s", [1, HS], DT, kind="ExternalInput")
    # per-matrix dequant scales (1.0 in bf16 mode): [s1, s2, sl, 0],
    # replicated across partitions so any column group can read them
    sc_ext = nc.dram_tensor("sc", [128, 4], DT, kind="ExternalInput")
    out_ext = nc.dram_tensor("out", [1, OS], DT, kind="ExternalOutput")

    NS = cfg["ns"]                      # per-gate split width (8 x 625)
    NSPLIT = len(cfg["gates"])
    h1b = nc.dram_tensor("h1b", [SEG], VDT)
    h2b = nc.dram_tensor("h2b", [SEG], VDT)
    g2p = [nc.dram_tensor(f"g2p{s}", [NS], VDT) for s in range(NSPLIT)]
    g2q = [nc.dram_tensor(f"g2q{s}", [NS], VDT) for s in range(NSPLIT)]
    warm_in = nc.dram_tensor("warm_in", [64], VDT)
    warm_out = nc.dram_tensor("warm_out", [64], VDT)
    opart = nc.dram_tensor("opart", [I_DIM], DT)
    oa2a = nc.dram_tensor("oa2a", [I_DIM], DT)

    groups = [list(range(N_CORES))]

    with tile.TileContext(nc) as tc:
        with (
            tc.tile_pool(name="wpool", bufs=4 if fast else 3) as wpool,
            tc.tile_pool(name="wlpool", bufs=1) as wlpool,
            tc.tile_pool(name="misc", bufs=1) as misc,
            tc.tile_pool(name="stage", bufs=2) as stpool,
            tc.tile_pool(name="psum", bufs=8, space="PSUM") as ppool,
        ):
            hwdge = [nc.sync, nc.scalar]
            small_i = 0

            def wdma(dst, src):
                # weight streams: SWDGE (casting) in i8 mode, HWDGE else
                nonlocal small_i
                if i8:
                    nc.gpsimd.dma_start(out=dst, in_=src)
                else:
                    hwdge[small_i % 2].dma_start(out=dst, in_=src)
                    small_i += 1

            def sdma(dst, src):
                # small transfers: HWDGE in i8 mode (queues are free),
                # SWDGE otherwise (HWDGE busy with weights)
                nonlocal small_i
                if i8:
                    hwdge[small_i % 2].dma_start(out=dst, in_=src)
                    small_i += 1
                else:
                    nc.gpsimd.dma_start(out=dst, in_=src)

            # --- small input DMAs ---
            vec1_sb = misc.tile([128, B1], VDT, name="vec1sb")
            nc.gpsimd.dma_start(out=vec1_sb[:], in_=vec1_ext[:])
            vec2_sb = misc.tile([128, B2], VDT, name="vec2sb")
            if not fast:
                nc.gpsimd.dma_start(out=vec2_sb[:, BL:B2], in_=h2ts_ext[:])
            vecl_sb = misc.tile([128, BL], VDT, name="veclsb")
            sc_sb = misc.tile([128, 4], DT, name="scsb")
            nc.gpsimd.dma_start(out=sc_sb[:], in_=sc_ext[:])
            ones_sb = misc.tile([N_CORES, 1], DT, name="onessb")
            nc.vector.memset(ones_sb[:], 1.0)
            ones_bf = misc.tile([N_CORES, 1], VDT, name="onesbf")
            nc.vector.memset(ones_bf[:], 1.0)
            # tiny warmup collective: pays the collectives-path first-use
            # cost (~15us) inside the init-barrier shadow
            warm_sb = misc.tile([1, 64], VDT, name="warmsb")
            nc.vector.memset(warm_sb[:], 0.0)
            nc.gpsimd.dma_start(out=warm_in.ap(), in_=warm_sb[0:1, :])
            nc.gpsimd.collective_compute(
                "AllToAll", mybir.AluOpType.bypass, replica_groups=groups,
                ins=[warm_in.ap().opt()], outs=[warm_out.ap().opt()])
            c1_sb = c2_sb = None
            if not fast:
                c1_sb = misc.tile([1, HS], DT, name="c1sb")
                c2_sb = misc.tile([1, HS], DT, name="c2sb")
                for i in range(5):
                    sl = slice(i * 125, (i + 1) * 125)
                    nc.gpsimd.dma_start(out=c1_sb[:, sl], in_=c1s_ext[:, sl])
                    nc.gpsimd.dma_start(out=c2_sb[:, sl], in_=c2s_ext[:, sl])

            def hmath(gates_sb, c_sb, hpad_sb):
                """c_new = sig(f)*c + i*g ; h = o * tanh(c_new)."""
                g = cfg["gates"]
                ap = lambda ch: gates_sb[:, g.index(ch) * HS:
                                         (g.index(ch) + 1) * HS]
                m1 = misc.tile([1, HS], DT, name="m1")
                nc.vector.tensor_mul(m1[:], ap("i"), ap("g"))
                if not fast:
                    m2 = misc.tile([1, HS], DT, name="m2")
                    nc.vector.tensor_mul(m2[:], ap("f"), c_sb[:])
                    nc.vector.tensor_add(m1[:], m1[:], m2[:])
                nc.scalar.activation(m1[:], m1[:], Tanh)
                nc.vector.tensor_mul(hpad_sb[:, 0:HS], ap("o"), m1[:])

            def to_vec(hpad_sb, dram, vdst):
                """[1, 640] partition-0 h vector -> [128, 5] vec layout,
                bounced through DRAM."""
                sdma(dram.ap(), hpad_sb[0:1, :])
                sdma(vdst, dram.ap().rearrange("(b p) -> p b", p=128))

            def warm_mms(wt, j, count):
                """Throwaway matmuls re-reading an already-loaded weight
                tile: they fill the PE's DMA-wait gap between tiles so the
                HAM clock gate never re-throttles (Q7f oscillation)."""
                pdx = ppool.tile([1, 512], DT, name="pdx", tag="ps")
                for _ in range(count):
                    nc.tensor.matmul(pdx[:, 0:512], vec1_sb[:, 0:1],
                                     wt[:, j, 0:512], start=True, stop=True)

            # =========== cell 1: column-sharded, all local ===========
            chunks1 = cfg["c1_chunks"]
            pg = [ppool.tile([1, 512], DT, name=f"pg{n}", tag="ps")
                  for n in range(len(chunks1))]
            # small first tile so the tensor engine starts early
            tiles1 = [(0, 2)] + [(b, min(b + 6, B1) - b) for b in range(2, B1, 6)]
            for ti, (b0, nb) in enumerate(tiles1):
                wt = wpool.tile([128, 6, GPC], VDT, tag="w", name="w1t")
                wdma(wt[:, 0:nb, :],
                     w1_ext[b0 * 128:(b0 + nb) * 128, :]
                     .rearrange("(n p) c -> p n c", p=128))
                for j in range(nb):
                    b = b0 + j
                    for n, (c0, c1) in enumerate(chunks1):
                        nc.tensor.matmul(
                            pg[n][:, 0:c1 - c0],
                            vec1_sb[:, b:b + 1],
                            wt[:, j, c0:c1],
                            start=(b == 0), stop=(b == B1 - 1))
                if ti < len(tiles1) - 1:
                    warm_mms(wt, 0, 12)
            gates1 = misc.tile([1, GPC], DT, name="gates1")
            for (ci, lo, hi, func, glo, ghi) in _act_map(cfg["gates"], chunks1):
                nc.scalar.activation(gates1[:, glo:ghi], pg[ci][:, lo:hi],
                                     func, scale=sc_sb[0:1, 0:1])
            h1pad = misc.tile([1, SEG], VDT, name="h1pad")
            nc.vector.memset(h1pad[:], 0.0)
            nc.vector.memset(h1pad[:, HS:HS + 1], 1.0)
            hmath(gates1, c1_sb, h1pad)
            to_vec(h1pad, h1b, vec2_sb[:, 0:BL])

            # w_lin prefetch: queued here (no buffer-reuse waits of its
            # own) so the gpsimd queue reaches the collective triggers
            # without waiting on it
            wl_sb = wlpool.tile([128, BL, I_DIM], VDT, name="wlsb")
            wdma(wl_sb[:],
                 wl_ext.ap().rearrange("(n p) c -> p n c", p=128))

            # ==== cell 2: contraction-sharded, partial sums over all G ====
            # Columns are laid out gate-major then rank-major, so each
            # per-gate group of NS columns feeds its own AllToAll as soon
            # as its tiles' stores land (the first collective's ~12us
            # setup hides behind the remaining weight stream).  Weight
            # tiles span all B2 k-blocks x a column window; chunks within
            # a tile are k-major (PSUM-bank-alternating) and staged out
            # as one batch store per tile.
            for t0 in range(0, G, CW):
                wt = wpool.tile([128, B2, CW], VDT, tag="w", name="w2t")
                wdma(wt[:],
                     w2_ext[:, t0:t0 + CW]
                     .rearrange("(n p) c -> p n c", p=128))
                tchunks = _chunks(CW)
                pc = [ppool.tile([1, 512], DT, name=f"pc{n}", tag="ps")
                      for n in range(len(tchunks))]
                for b in range(B2):
                    for n, (c0, c1) in enumerate(tchunks):
                        nc.tensor.matmul(
                            pc[n][:, 0:c1 - c0],
                            vec2_sb[:, b:b + 1],
                            wt[:, b, c0:c1],
                            start=(b == 0), stop=(b == B2 - 1))
                st = stpool.tile([1, CW], VDT, name="st", tag="st")
                for n, (c0, c1) in enumerate(tchunks):
                    nc.scalar.activation(st[:, c0:c1], pc[n][:, 0:c1 - c0],
                                         Copy, scale=sc_sb[0:1, 1:2])
                sdma(g2p[t0 // NS][t0 % NS:t0 % NS + CW], st[0:1, :])
                warm_mms(wt, 0, 7)

            # --- per-gate AllToAll + local ones-matmul reduction; the
            # c/h elementwise math interleaves between splits ---
            g2a = misc.tile([1, GPC], DT, name="g2a")
            h2pad = misc.tile([1, SEG], VDT, name="h2pad")
            nc.vector.memset(h2pad[:], 0.0)
            nc.vector.memset(h2pad[:, HS:HS + 1], 1.0)
            m1 = misc.tile([1, HS], DT, name="m1c2")
            funcs = {"i": Sig, "f": Sig, "g": Tanh, "o": Sig}
            for s, gname in enumerate(cfg["gates"]):
                nc.gpsimd.collective_compute(
                    "AllToAll", mybir.AluOpType.bypass, replica_groups=groups,
                    ins=[g2p[s].ap().opt()], outs=[g2q[s].ap().opt()])
                parts = misc.tile([N_CORES, HS], VDT, name=f"parts{s}")
                sdma(parts[:], g2q[s].ap().rearrange("(q e) -> q e", q=N_CORES))
                ga = g2a[:, s * HS:(s + 1) * HS]
                for (c0, c1) in _chunks(HS):
                    pr = ppool.tile([1, 512], DT, name=f"pr{s}", tag="ps")
                    nc.tensor.matmul(pr[:, 0:c1 - c0], ones_bf[:],
                                     parts[:, c0:c1], start=True, stop=True)
                    nc.scalar.activation(ga[:, c0:c1], pr[:, 0:c1 - c0],
                                         funcs[gname])
                # interleaved h math once inputs are ready
                g = cfg["gates"]
                if gname == "g":
                    ap = lambda ch: g2a[:, g.index(ch) * HS:
                                        (g.index(ch) + 1) * HS]
                    nc.vector.tensor_mul(m1[:], ap("i"), ap("g"))
                    if not fast:
                        m2 = misc.tile([1, HS], DT, name="m2c2")
                        nc.vector.tensor_mul(m2[:], ap("f"), c2_sb[:])
                        nc.vector.tensor_add(m1[:], m1[:], m2[:])
                    nc.scalar.activation(m1[:], m1[:], Tanh)
                if gname == "o":
                    o_ap = g2a[:, g.index("o") * HS:(g.index("o") + 1) * HS]
                    nc.vector.tensor_mul(h2pad[:, 0:HS], o_ap, m1[:])
            to_vec(h2pad, h2b, vecl_sb[:])

            # ===== final linear: contraction-sharded, SBUF-resident =====
            # a few throwaway matmuls keep the PE clock warm through the
            # collective wait so the final matvecs run at full rate
            pdum = ppool.tile([1, 512], DT, name="pdum", tag="ps")
            for _ in range(12):
                nc.tensor.matmul(pdum[:, 0:512], wl_sb[:, 0, 0:1],
                                 wl_sb[:, 1, 0:512], start=True, stop=True)
            # out chunks ride different PE column groups (tile_position)
            # so each k-block's 8 matvecs run as two concurrent waves; the
            # partials need no elementwise math, so the partition spread
            # costs nothing (per-chunk staging copies + stores).
            chunksl = cfg["cl_chunks"]
            pl = [ppool.tile([128, 512], DT, name=f"pl{h}", tag="ps")
                  for h in range((len(chunksl) + 3) // 4)]
            for b in range(BL):
                for n, (c0, c1) in enumerate(chunksl):
                    cg = 32 * (n % 4)
                    nc.tensor.matmul(
                        pl[n // 4][cg:cg + 1, 0:c1 - c0],
                        vecl_sb[:, b:b + 1],
                        wl_sb[:, b, c0:c1],
                        start=(b == 0), stop=(b == BL - 1),
                        tile_position=(0, cg))
            stl = stpool.tile([128, 1024], DT, name="stl", tag="stl", bufs=1)
            for n, (c0, c1) in enumerate(chunksl):
                cg = 32 * (n % 4)
                co = (n // 4) * 512
                nc.scalar.activation(stl[cg:cg + 1, co:co + c1 - c0],
                                     pl[n // 4][cg:cg + 1, 0:c1 - c0],
                                     Copy, scale=sc_sb[cg:cg + 1, 2:3])
                sdma(opart[c0:c1], stl[cg:cg + 1, co:co + c1 - c0])

            # --- AllToAll + local reduction -> this core's output slice ---
            nc.gpsimd.collective_compute(
                "AllToAll", mybir.AluOpType.bypass, replica_groups=groups,
                ins=[opart.ap().opt()], outs=[oa2a.ap().opt()])
            partsl = misc.tile([N_CORES, OS], DT, name="partsl")
            sdma(partsl[:], oa2a.ap().rearrange("(q e) -> q e", q=N_CORES))
            po = ppool.tile([1, 512], DT, name="po", tag="ps")
            nc.tensor.matmul(po[:, 0:OS], ones_sb[:], partsl[:],
                             start=True, stop=True)
            out_sb = misc.tile([1, OS], DT, name="outsb")
            nc.scalar.activation(out_sb[:], po[:, 0:OS], Copy)
            nc.sync.dma_start(out=out_ext[:], in_=out_sb[0:1, :])

    nc.compile()
    return nc, cfg


def _quant(w):
    """Symmetric int8 quantization; returns (stored array, descale)."""
    if WMODE == "i8":
        s = float(np.abs(w).max()) / 127.0
        if s == 0.0:
            s = 1.0
        return np.round(w / s).astype(np.int8), s
    return w.astype(BF16), 1.0


def _gate_cols(w, r, gates):
    """[in_dim, gpc] column block for core r (gate-major), transposed so
    rows are the contraction dim."""
    gidx = {"i": 0, "f": 1, "g": 2, "o": 3}
    outb = np.empty((w.shape[1], len(gates) * HS), dtype=F32)
    for k, gname in enumerate(gates):
        rows = slice(gidx[gname] * H_DIM + r * HS,
                     gidx[gname] * H_DIM + (r + 1) * HS)
        outb[:, k * HS:(k + 1) * HS] = w[rows, :].T
    return outb


def _gate_bias(b_a, b_b, r, gates):
    gidx = {"i": 0, "f": 1, "g": 2, "o": 3}
    out = np.empty((len(gates) * HS,), dtype=F32)
    for k, gname in enumerate(gates):
        rows = slice(gidx[gname] * H_DIM + r * HS,
                     gidx[gname] * H_DIM + (r + 1) * HS)
        out[k * HS:(k + 1) * HS] = b_a[rows] + b_b[rows]
    return out


def _perm_gate_major(a, ng):
    """[..., 8*ng*HS] rank-major -> gate-major (gate, rank, elem) layout."""
    shp = a.shape[:-1]
    a = a.reshape(shp + (N_CORES, ng, HS))
    a = np.moveaxis(a, -3, -2)
    return np.ascontiguousarray(a.reshape(shp + (N_CORES * ng * HS,)))


def _prep_shared(fast, cfg, args):
    """Host-side tensors shared across cores (full cell-2 / w_lin column
    panels, gate-major then rank-major; sliced by contraction rows per
    core)."""
    gates = cfg["gates"]
    ng = len(gates)
    cols2 = _perm_gate_major(np.concatenate(
        [_gate_cols(args["w_ih2"], q, gates) for q in range(N_CORES)],
        axis=1), ng)
    bias2 = _perm_gate_major(np.concatenate(
        [_gate_bias(args["b_ih2"], args["b_hh2"], q, gates)
         for q in range(N_CORES)]), ng)
    colsh2 = None
    if not fast:
        colsh2 = _perm_gate_major(np.concatenate(
            [_gate_cols(args["w_hh2"], q, gates) for q in range(N_CORES)],
            axis=1), ng)
    return dict(cols2=cols2, bias2=bias2, colsh2=colsh2,
                wlT=args["w_lin"].T.astype(F32))


def _prep_core(r, fast, cfg, shared, input_data, w_ih1, w_hh1, b_ih1, b_hh1,
               w_ih2, w_hh2, b_ih2, b_hh2, w_lin, b_lin,
               h_t, c_t, h2_t, c2_t):
    gates, GPC, G = cfg["gates"], cfg["gpc"], cfg["G"]

    # --- W1 (column-sharded): [x-seg | (h-seg)] x GPC ---
    w1 = np.zeros((cfg["r1"], GPC), dtype=F32)
    w1[0:I_DIM] = _gate_cols(w_ih1, r, gates)
    w1[I_DIM] = _gate_bias(b_ih1, b_hh1, r, gates)
    if not fast:
        w1[XSEG:XSEG + H_DIM] = _gate_cols(w_hh1, r, gates)

    # --- W2 (contraction-sharded): [own h1 rows | (own h2_t rows)] x G ---
    w2 = np.zeros((cfg["r2"], G), dtype=F32)
    w2[0:HS] = shared["cols2"][r * HS:(r + 1) * HS]
    if r == 0:
        w2[HS] = shared["bias2"]          # rides the 1.0 slot, core 0 only
    if not fast:
        w2[SEG:SEG + HS] = shared["colsh2"][r * HS:(r + 1) * HS]

    # --- W_lin (contraction-sharded): [own h2 rows] x I_DIM ---
    wl = np.zeros((SEG, I_DIM), dtype=F32)
    wl[0:HS] = shared["wlT"][r * HS:(r + 1) * HS]
    if r == 0:
        wl[HS] = b_lin

    w1, s1 = _quant(w1)
    w2, s2 = _quant(w2)
    wl, sl = _quant(wl)

    vec1 = np.zeros((cfg["r1"],), dtype=BF16)
    vec1[0:I_DIM] = input_data[0]
    vec1[I_DIM] = 1.0
    if not fast:
        vec1[XSEG:XSEG + H_DIM] = h_t[0]
    vec1 = np.ascontiguousarray(vec1.reshape(cfg["r1"] // 128, 128).T)

    m = {
        "w1": w1, "w2": w2, "wl": wl, "vec1": vec1,
        "sc": np.tile(np.array([[s1, s2, sl, 0.0]], dtype=F32), (128, 1)),
    }
    if not fast:
        h2ts = np.zeros((SEG,), dtype=BF16)
        h2ts[0:HS] = h2_t[0, r * HS:(r + 1) * HS]
        m["h2ts"] = np.ascontiguousarray(h2ts.reshape(SEG // 128, 128).T)
        m["c1s"] = np.ascontiguousarray(c_t[:, r * HS:(r + 1) * HS], dtype=F32)
        m["c2s"] = np.ascontiguousarray(c2_t[:, r * HS:(r + 1) * HS], dtype=F32)
    return m


def kernel(**inputs):
    args = {k: np.asarray(v, dtype=F32) for k, v in inputs.items()}
    fast = not any(np.any(args[k]) for k in ("h_t", "c_t", "h2_t", "c2_t"))

    if fast not in _CACHED:
        _CACHED[fast] = _build_bass(fast)
    nc, cfg = _CACHED[fast]

    shared = _prep_shared(fast, cfg, args)
    in_maps = [_prep_core(r, fast, cfg, shared, **args) for r in range(N_CORES)]
    res = run_bass_kernel_spmd(nc, in_maps, core_ids=list(range(N_CORES)))
    out = np.concatenate([res.results[r]["out"][0] for r in range(N_CORES)])
    return out.reshape(1, I_DIM).astype(np.float32)
